# revision 4
# baseline (speedup 1.0000x reference)
"""Causal self-attention (B=2, T=2048, E=1024, H=16) on 8 trn2 NeuronCores.

Sharding: tensor-parallel over heads — core c owns heads {2c, 2c+1}.
Each core:
  1. qkv projection for its heads:  Q^T/K^T in [d, t] layout (d on
     partitions), V via PE-transpose into [t, d] layout.  For b=0 the
     ct loop is outermost (4 psum accumulators) so matmuls start as
     each x tile lands from HBM; b=1 runs chunk-outer on 2 banks since
     its x load is hidden under b=0's attention.
  2. causal attention with scores TRANSPOSED ([k, q] layout), processed
     in 512-query windows:
       scoresT(h0)/scoresT(h1) land in one [128, 2, 512] fp32 PSUM tile
       (2 banks); the two score matmuls are 64x128 row-tiles (T0 / T8)
       that the PE runs concurrently;
       ONE exp ACTIVATE per k-tile covers both heads;
       probsT  = exp * causal band mask (one paired mul, diagonal tiles)
       outT   += matmul(lhsT=V_aug, rhs=probsT)  (V_aug has a ones
                 column; its output row is the softmax denominator l)
       out     = outT[0:64] * (1/l)
  3. output projection emitted per 512-token window so stores overlap
     the next window's compute; partials stored in bf16, summed on the
     host.

PSUM budget (8 banks): 2x scores [128,2,512] fp32 (4) + pv pool 2x
[128,512]-slot (2; holds the [65,512] PV accumulators and the outproj
psum) + 2x proj slots (2; proj chunks and V transposes).  proj and
outproj use different pools so batch 1's projection is not queued
behind batch 0's outproj by pool-slot rotation.

The softmax 1/l uses the exact DVE reciprocal on a DMA-reshaped
[128, 4] layout (SWDGE queue), then partition-broadcast on GpSimd.
"""

import numpy as np
import ml_dtypes
from contextlib import ExitStack

import concourse.bass as bass
import concourse.mybir as mybir
import concourse.tile as tile
from concourse import bacc
from concourse.bass_utils import run_bass_kernel_spmd
from concourse.masks import make_identity

B, T, E, H, D = 2, 2048, 1024, 16, 64
NCORES = 8
HPC = H // NCORES          # heads per core = 2
JC = HPC * D               # local out-projection columns per core = 128
W = 512                    # query window (one PSUM bank of fp32)
KT = 128                   # k tile (matmul M limit)
NW = T // W                # windows per head per batch = 4
NKT = T // KT              # k tiles = 16

BF16 = mybir.dt.bfloat16
FP32 = mybir.dt.float32
NPBF = ml_dtypes.bfloat16
EXP = mybir.ActivationFunctionType.Exp

_NC_CACHE = []


def _build_nc():
    nc = bacc.Bacc(None, target_bir_lowering=False)

    xT = nc.dram_tensor("xT", [E, B, T], BF16, kind="ExternalInput")
    wqkvT = nc.dram_tensor("wqkvT", [E, 3 * JC], BF16, kind="ExternalInput")
    wpT = nc.dram_tensor("wpT", [JC, E], BF16, kind="ExternalInput")
    outp = nc.dram_tensor("outp", [B, T, E], BF16, kind="ExternalOutput")

    with tile.TileContext(nc) as tc, ExitStack() as ctx:
        const_pool = ctx.enter_context(tc.tile_pool(name="const", bufs=1))
        w_pool = ctx.enter_context(tc.tile_pool(name="w", bufs=1))
        xt_pool = ctx.enter_context(tc.tile_pool(name="xt", bufs=2))
        qk_pool = ctx.enter_context(tc.tile_pool(name="qk", bufs=2))
        va_pool = ctx.enter_context(tc.tile_pool(name="va", bufs=2))
        vtmp_pool = ctx.enter_context(tc.tile_pool(name="vtmp", bufs=2))
        probs_pool = ctx.enter_context(tc.tile_pool(name="probs", bufs=6))
        outT_pool = ctx.enter_context(tc.tile_pool(name="outT", bufs=2))
        norm_pool = ctx.enter_context(tc.tile_pool(name="norm", bufs=4))
        stage_pool = ctx.enter_context(tc.tile_pool(name="stage", bufs=3))
        sc_ps = ctx.enter_context(tc.tile_pool(name="sc_ps", bufs=2, space="PSUM"))
        pv_ps = ctx.enter_context(tc.tile_pool(name="pv_ps", bufs=2, space="PSUM"))
        pj_ps = ctx.enter_context(tc.tile_pool(name="pj_ps", bufs=2, space="PSUM"))

        # --- constants -------------------------------------------------
        ident = const_pool.tile([128, 128], BF16)
        make_identity(nc, ident[:])

        # mask2[p, h, j] = 1 iff j >= p (causal band, copy per head)
        mask2 = const_pool.tile([128, HPC, KT], BF16)
        nc.gpsimd.memset(mask2[:], 1.0)
        nc.gpsimd.affine_select(
            out=mask2[:],
            in_=mask2[:],
            compare_op=mybir.AluOpType.is_ge,
            fill=0.0,
            base=0,
            channel_multiplier=-1,
            pattern=[[0, HPC], [1, KT]],
        )

        # --- weights + batch-0 x, loads interleaved --------------------
        wq_sb = [
            w_pool.tile([128, 3 * JC], BF16, tag=f"wq{i}", name=f"wq{i}")
            for i in range(8)
        ]
        wp_sb = w_pool.tile([JC, E], BF16, tag="wp")
        nc.sync.dma_start(wp_sb[:], wpT[:])

        for b in range(B):
            xt = [
                xt_pool.tile([128, T], BF16, tag=f"xt{i}", name=f"xt{i}")
                for i in range(8)
            ]
            for i in range(8):
                if b == 0:
                    nc.sync.dma_start(wq_sb[i][:], wqkvT[i * 128 : (i + 1) * 128, :])
                nc.sync.dma_start(xt[i][:], xT[i * 128 : (i + 1) * 128, b, :])

            QT = qk_pool.tile([128, T], BF16, tag="QT")
            KTs = qk_pool.tile([128, T], BF16, tag="KT")
            vaug = va_pool.tile([128, NKT, HPC, D + 1], BF16, tag="va")
            nc.gpsimd.memset(vaug[:, :, :, D : D + 1], 1.0)

            vtmps = [
                vtmp_pool.tile([128, W], BF16, tag=f"vtmp{c}", name=f"vtmp{c}")
                for c in range(4)
            ]

            def proj_out(fb, c, pp):
                tsl = slice(c * W, (c + 1) * W)
                if fb == 0:
                    nc.vector.tensor_copy(QT[:, tsl], pp[:])
                elif fb == 1:
                    nc.vector.tensor_copy(KTs[:, tsl], pp[:])
                else:
                    nc.vector.tensor_copy(vtmps[c][:], pp[:])

            if b == 0:
                # ct-outer: all 4 chunks accumulate as each xt DMA lands
                for fb in (2, 0, 1):  # V, Q, K
                    pp = [
                        (sc_ps if c < 2 else pj_ps).tile(
                            [128, W], FP32, tag="sc_ps" if c < 2 else "pj_ps",
                            name=f"pp{c}",
                        )
                        for c in range(4)
                    ]
                    for ct in range(8):
                        for c in range(4):
                            nc.tensor.matmul(
                                pp[c][:],
                                wq_sb[ct][:, fb * 128 : (fb + 1) * 128],
                                xt[ct][:, c * W : (c + 1) * W],
                                start=(ct == 0),
                                stop=(ct == 7),
                            )
                    for c in range(4):
                        proj_out(fb, c, pp[c])
                for kt_idx in range(16):
                    c, sub = divmod(kt_idx, W // KT)
                    ptr = pj_ps.tile([128, HPC, D], BF16, tag="pj_ps", name="ptr")
                    nc.tensor.transpose(
                        ptr[:], vtmps[c][:, sub * KT : (sub + 1) * KT], ident[:]
                    )
                    nc.vector.tensor_copy(vaug[:, kt_idx, :, 0:D], ptr[:])
            else:
                # chunk-outer on the 2-slot proj pool (x already loaded)
                for fb in (2, 0, 1):  # V, Q, K
                    for c in range(4):
                        pp = pj_ps.tile([128, W], FP32, tag="pj_ps", name="pp")
                        for ct in range(8):
                            nc.tensor.matmul(
                                pp[:],
                                wq_sb[ct][:, fb * 128 : (fb + 1) * 128],
                                xt[ct][:, c * W : (c + 1) * W],
                                start=(ct == 0),
                                stop=(ct == 7),
                            )
                        proj_out(fb, c, pp)
                    if fb == 2:
                        for kt_idx in range(16):
                            c, sub = divmod(kt_idx, W // KT)
                            ptr = pj_ps.tile(
                                [128, HPC, D], BF16, tag="pj_ps", name="ptr"
                            )
                            nc.tensor.transpose(
                                ptr[:],
                                vtmps[c][:, sub * KT : (sub + 1) * KT],
                                ident[:],
                            )
                            nc.vector.tensor_copy(vaug[:, kt_idx, :, 0:D], ptr[:])

            # --- attention, 512-query windows, both heads together ----
            outTt = outT_pool.tile([128, T], BF16, tag="outT")
            for wi in range(NW):
                q0 = wi * W
                nkt = (q0 + W) // KT
                qsl0 = slice(q0, q0 + W)

                ops = [
                    pv_ps.tile([D + 1, W], FP32, tag="pv_ps", name=f"ops{h}")
                    for h in range(HPC)
                ]

                def scores_exp(kt):
                    koff = kt * KT - q0
                    lo = max(0, koff)
                    ksl = slice(kt * KT, (kt + 1) * KT)
                    S = sc_ps.tile([128, HPC, W], FP32, tag="sc_ps", name="S")
                    pr = probs_pool.tile([128, HPC, W], BF16, tag="probs")
                    # the two heads' score matmuls are 64x128 row tiles
                    # (T0 and T8) — the PE runs them concurrently
                    for h in range(HPC):
                        po = h * D
                        nc.tensor.matmul(
                            S[:, h, lo:W],
                            KTs[po : po + D, ksl],
                            QT[po : po + D, q0 + lo : q0 + W],
                            start=True,
                            stop=True,
                        )
                    nc.scalar.activation(pr[:, :, lo:W], S[:, :, lo:W], EXP)
                    if koff >= 0:  # diagonal tile: mask both heads' bands
                        nc.vector.tensor_mul(
                            pr[:, :, lo : lo + KT], pr[:, :, lo : lo + KT], mask2[:]
                        )
                    return pr, lo

                def pv(kt, pr, lo):
                    for h in range(HPC):
                        nc.tensor.matmul(
                            ops[h][:, lo:W],
                            vaug[:, kt, h, :],
                            pr[:, h, lo:W],
                            start=(kt == 0),
                            stop=(kt == nkt - 1),
                        )

                # software pipeline: PV trails scores by two k-tiles so
                # the PE never waits on ACT's exp.
                pending = []
                for kt in range(nkt):
                    pending.append((kt, scores_exp(kt)))
                    if len(pending) > 2:
                        k0, (pr0, lo0) = pending.pop(0)
                        pv(k0, pr0, lo0)
                for k0, (pr0, lo0) in pending:
                    pv(k0, pr0, lo0)

                # normalize: out = ops[0:D] / l,  l = ops[D].
                for h in range(HPC):
                    po = h * D
                    lrow = norm_pool.tile([1, W], FP32, tag="lrow")
                    nc.vector.tensor_copy(lrow[:], ops[h][D : D + 1, :])
                    lT = norm_pool.tile([128, W // 128], FP32, tag="lT")
                    nc.gpsimd.dma_start(lT[:], lrow[:])
                    rT = norm_pool.tile([128, W // 128], FP32, tag="rT")
                    nc.vector.reciprocal(rT[:], lT[:])
                    rrow = norm_pool.tile([1, W], FP32, tag="rrow")
                    nc.gpsimd.dma_start(rrow[:], rT[:])
                    bc = norm_pool.tile([D, W], FP32, tag="bc")
                    nc.gpsimd.partition_broadcast(bc[:], rrow[:])
                    nc.vector.tensor_mul(
                        outTt[po : po + D, qsl0], ops[h][0:D, :], bc[:]
                    )

                # --- output projection for this window's 4 t-blocks ---
                for tb in range(q0 // KT, (q0 + W) // KT):
                    st = stage_pool.tile([128, E], BF16, tag="stage")
                    for oc in range(2):
                        pj = pv_ps.tile([128, W], FP32, tag="pv_ps", name="pj")
                        nc.tensor.matmul(
                            pj[:],
                            outTt[:, tb * KT : (tb + 1) * KT],
                            wp_sb[:, oc * W : (oc + 1) * W],
                            start=True,
                            stop=True,
                        )
                        nc.vector.tensor_copy(st[:, oc * W : (oc + 1) * W], pj[:])
                    nc.sync.dma_start(outp[b, tb * KT : (tb + 1) * KT, :], st[:])

    nc.compile()
    return nc


def _get_nc():
    if not _NC_CACHE:
        _NC_CACHE.append(_build_nc())
    return _NC_CACHE[0]


def make_in_maps(x, w_qkv, w_proj):
    x = np.asarray(x, np.float32)
    w_qkv = np.asarray(w_qkv, np.float32)
    w_proj = np.asarray(w_proj, np.float32)
    xT = np.ascontiguousarray(x.transpose(2, 0, 1)).astype(NPBF)  # [E, B, T]
    in_maps = []
    for c in range(NCORES):
        h0 = c * HPC
        wq = w_qkv[h0 * D : (h0 + HPC) * D] * 0.125  # fold softmax scale
        wk = w_qkv[E + h0 * D : E + (h0 + HPC) * D]
        wv = w_qkv[2 * E + h0 * D : 2 * E + (h0 + HPC) * D]
        wqkvT = np.ascontiguousarray(np.concatenate([wq, wk, wv], 0).T)
        wpTc = np.ascontiguousarray(w_proj[:, c * JC : (c + 1) * JC].T)
        in_maps.append(
            {
                "xT": xT,
                "wqkvT": wqkvT.astype(NPBF),
                "wpT": wpTc.astype(NPBF),
            }
        )
    return in_maps


def kernel(x, w_qkv, w_proj, **run_kwargs):
    in_maps = make_in_maps(x, w_qkv, w_proj)
    nc = _get_nc()
    res = run_bass_kernel_spmd(nc, in_maps, core_ids=list(range(NCORES)), **run_kwargs)
    out = res.results[0]["outp"].astype(np.float32)
    for r in res.results[1:]:
        out += r["outp"].astype(np.float32)
    if run_kwargs:
        kernel.last_results = res
    return out


# revision 5
# speedup vs baseline: 1.0486x; 1.0486x over previous
"""Causal self-attention (B=2, T=2048, E=1024, H=16) on 8 trn2 NeuronCores.

Sharding: tensor-parallel over heads — core c owns heads {2c, 2c+1}.
Each core:
  1. qkv projection for its heads:  Q^T/K^T in [d, t] layout (d on
     partitions), V via PE-transpose into [t, d] layout.  For b=0 the
     ct loop is outermost (4 psum accumulators, 2 borrowed from the
     scores pool) so matmuls start as each x tile lands from HBM; b=1
     runs chunk-outer on the 2-slot proj pool since its x load hides
     under b=0's attention.
  2. causal attention with scores TRANSPOSED ([k, q] layout), processed
     in 512-query windows:
       scoresT(h0)/scoresT(h1) land in one [128, 2, 512] fp32 PSUM tile
       (2 banks); the two score matmuls are 64x128 row-tiles (T0 / T8)
       that the PE runs concurrently;
       ONE exp ACTIVATE per k-tile covers both heads;
       probsT  = exp * causal band mask (one paired mul on diag tiles)
       outT   += matmul(lhsT=V_aug, rhs=probsT)  (V_aug has a ones
                 column; its output row is the softmax denominator l)
       out     = outT[0:64] * (1/l)     (both heads share one
                 DMA-reshape / reciprocal / partition-broadcast chain)
  3. output projection: batch 0's is emitted AFTER batch 1's projection
     so the psum-slot rotation schedules it into batch 1's attention;
     batch 1's is emitted per window.  Partials are stored bf16 and
     summed on the host.

PSUM budget (8 banks): 2x scores [128,2,512] fp32 (4) + 2x [65,512] PV
accumulators (2) + 2x proj/outproj/transpose slots (2).  The PV pool
serves ONLY the accumulators: pool slots rotate in tile-creation order,
so sharing a pool across phases serializes the later phase behind the
earlier one.
"""

import numpy as np
import ml_dtypes
from contextlib import ExitStack

import concourse.bass as bass
import concourse.mybir as mybir
import concourse.tile as tile
from concourse import bacc
from concourse.bass_utils import run_bass_kernel_spmd
from concourse.masks import make_identity

B, T, E, H, D = 2, 2048, 1024, 16, 64
NCORES = 8
HPC = H // NCORES          # heads per core = 2
JC = HPC * D               # local out-projection columns per core = 128
W = 512                    # query window (one PSUM bank of fp32)
KT = 128                   # k tile (matmul M limit)
NW = T // W                # windows per head per batch = 4
NKT = T // KT              # k tiles = 16

BF16 = mybir.dt.bfloat16
FP32 = mybir.dt.float32
NPBF = ml_dtypes.bfloat16
EXP = mybir.ActivationFunctionType.Exp

_NC_CACHE = []


def _build_nc():
    nc = bacc.Bacc(None, target_bir_lowering=False)

    xT = nc.dram_tensor("xT", [E, B, T], BF16, kind="ExternalInput")
    wqkvT = nc.dram_tensor("wqkvT", [E, 3 * JC], BF16, kind="ExternalInput")
    wpT = nc.dram_tensor("wpT", [JC, E], BF16, kind="ExternalInput")
    outp = nc.dram_tensor("outp", [B, T, E], BF16, kind="ExternalOutput")

    with tile.TileContext(nc) as tc, ExitStack() as ctx:
        const_pool = ctx.enter_context(tc.tile_pool(name="const", bufs=1))
        w_pool = ctx.enter_context(tc.tile_pool(name="w", bufs=1))
        xt_pool = ctx.enter_context(tc.tile_pool(name="xt", bufs=2))
        qk_pool = ctx.enter_context(tc.tile_pool(name="qk", bufs=2))
        va_pool = ctx.enter_context(tc.tile_pool(name="va", bufs=2))
        vtmp_pool = ctx.enter_context(tc.tile_pool(name="vtmp", bufs=2))
        probs_pool = ctx.enter_context(tc.tile_pool(name="probs", bufs=8))
        outT_pool = ctx.enter_context(tc.tile_pool(name="outT", bufs=2))
        norm_pool = ctx.enter_context(tc.tile_pool(name="norm", bufs=4))
        stage_pool = ctx.enter_context(tc.tile_pool(name="stage", bufs=3))
        sc_ps = ctx.enter_context(tc.tile_pool(name="sc_ps", bufs=2, space="PSUM"))
        pv_ps = ctx.enter_context(tc.tile_pool(name="pv_ps", bufs=2, space="PSUM"))
        pj_ps = ctx.enter_context(tc.tile_pool(name="pj_ps", bufs=2, space="PSUM"))

        # --- constants -------------------------------------------------
        ident = const_pool.tile([128, 128], BF16)
        make_identity(nc, ident[:])

        # mask2[p, h, j] = 1 iff j >= p (causal band, copy per head)
        mask2 = const_pool.tile([128, HPC, KT], BF16)
        nc.gpsimd.memset(mask2[:], 1.0)
        nc.gpsimd.affine_select(
            out=mask2[:],
            in_=mask2[:],
            compare_op=mybir.AluOpType.is_ge,
            fill=0.0,
            base=0,
            channel_multiplier=-1,
            pattern=[[0, HPC], [1, KT]],
        )

        wq_sb = [
            w_pool.tile([128, 3 * JC], BF16, tag=f"wq{i}", name=f"wq{i}")
            for i in range(8)
        ]
        wp_sb = w_pool.tile([JC, E], BF16, tag="wp")
        nc.sync.dma_start(wp_sb[:], wpT[:])

        # vaug / outT for both batches upfront; memsets run before any
        # partition_broadcast so the gpsimd library loads only once.
        vaugs = [
            va_pool.tile([128, NKT, HPC, D + 1], BF16, tag="va", name=f"va{b}")
            for b in range(B)
        ]
        for b in range(B):
            nc.gpsimd.memset(vaugs[b][:, :, :, D : D + 1], 1.0)
        outTs = [
            outT_pool.tile([128, T], BF16, tag="outT", name=f"outT{b}")
            for b in range(B)
        ]
        qks = []

        def emit_loads(b):
            xt = [
                xt_pool.tile([128, T], BF16, tag=f"xt{i}", name=f"xt{i}")
                for i in range(8)
            ]
            for i in range(8):
                if b == 0:
                    nc.sync.dma_start(wq_sb[i][:], wqkvT[i * 128 : (i + 1) * 128, :])
                nc.sync.dma_start(xt[i][:], xT[i * 128 : (i + 1) * 128, b, :])
            return xt

        def emit_proj(b, xt):
            QT = qk_pool.tile([128, T], BF16, tag="QT", name=f"QT{b}")
            KTs = qk_pool.tile([128, T], BF16, tag="KT", name=f"KT{b}")
            qks.append((QT, KTs))
            vaug = vaugs[b]
            vtmps = [
                vtmp_pool.tile([128, W], BF16, tag=f"vtmp{c}", name=f"vtmp{c}")
                for c in range(4)
            ]

            def proj_out(fb, c, pp):
                tsl = slice(c * W, (c + 1) * W)
                if fb == 0:
                    nc.vector.tensor_copy(QT[:, tsl], pp[:])
                elif fb == 1:
                    nc.vector.tensor_copy(KTs[:, tsl], pp[:])
                else:
                    nc.vector.tensor_copy(vtmps[c][:], pp[:])

            def transposes():
                for kt_idx in range(16):
                    c, sub = divmod(kt_idx, W // KT)
                    ptr = pj_ps.tile([128, HPC, D], BF16, tag="pj_ps", name="ptr")
                    nc.tensor.transpose(
                        ptr[:], vtmps[c][:, sub * KT : (sub + 1) * KT], ident[:]
                    )
                    nc.vector.tensor_copy(vaug[:, kt_idx, :, 0:D], ptr[:])

            if b == 0:
                # ct-outer: all 4 chunks accumulate as each xt DMA lands
                for fb in (2, 0, 1):  # V, Q, K
                    pp = [
                        (sc_ps if c < 2 else pj_ps).tile(
                            [128, W], FP32, tag="sc_ps" if c < 2 else "pj_ps",
                            name=f"pp{c}",
                        )
                        for c in range(4)
                    ]
                    for ct in range(8):
                        for c in range(4):
                            nc.tensor.matmul(
                                pp[c][:],
                                wq_sb[ct][:, fb * 128 : (fb + 1) * 128],
                                xt[ct][:, c * W : (c + 1) * W],
                                start=(ct == 0),
                                stop=(ct == 7),
                            )
                    for c in range(4):
                        proj_out(fb, c, pp[c])
                transposes()
            else:
                for fb in (2, 0, 1):  # V, Q, K
                    for c in range(4):
                        pp = pj_ps.tile([128, W], FP32, tag="pj_ps", name="pp")
                        for ct in range(8):
                            nc.tensor.matmul(
                                pp[:],
                                wq_sb[ct][:, fb * 128 : (fb + 1) * 128],
                                xt[ct][:, c * W : (c + 1) * W],
                                start=(ct == 0),
                                stop=(ct == 7),
                            )
                        proj_out(fb, c, pp)
                    if fb == 2:
                        transposes()

        def emit_outproj_tb(b, tb):
            st = stage_pool.tile([128, E], BF16, tag="stage")
            for oc in range(2):
                pj = pj_ps.tile([128, W], FP32, tag="pj_ps", name="pj")
                nc.tensor.matmul(
                    pj[:],
                    outTs[b][:, tb * KT : (tb + 1) * KT],
                    wp_sb[:, oc * W : (oc + 1) * W],
                    start=True,
                    stop=True,
                )
                nc.vector.tensor_copy(st[:, oc * W : (oc + 1) * W], pj[:])
            nc.sync.dma_start(outp[b, tb * KT : (tb + 1) * KT, :], st[:])

        def emit_attention(b, inline_outproj):
            QT, KTs = qks[b]
            vaug = vaugs[b]
            outTt = outTs[b]
            for wi in range(NW):
                q0 = wi * W
                nkt = (q0 + W) // KT
                qsl0 = slice(q0, q0 + W)

                ops = [
                    pv_ps.tile([D + 1, W], FP32, tag="pv_ps", name=f"ops{h}")
                    for h in range(HPC)
                ]

                def scores_exp(kt):
                    koff = kt * KT - q0
                    lo = max(0, koff)
                    ksl = slice(kt * KT, (kt + 1) * KT)
                    S = sc_ps.tile([128, HPC, W], FP32, tag="sc_ps", name="S")
                    pr = probs_pool.tile([128, HPC, W], BF16, tag="probs")
                    # 64x128 row tiles T0 / T8 — PE runs both heads
                    # concurrently
                    for h in range(HPC):
                        po = h * D
                        nc.tensor.matmul(
                            S[:, h, lo:W],
                            KTs[po : po + D, ksl],
                            QT[po : po + D, q0 + lo : q0 + W],
                            start=True,
                            stop=True,
                        )
                    nc.scalar.activation(pr[:, :, lo:W], S[:, :, lo:W], EXP)
                    if koff >= 0:
                        nc.vector.tensor_mul(
                            pr[:, :, lo : lo + KT], pr[:, :, lo : lo + KT], mask2[:]
                        )
                    return pr, lo

                def pv(kt, pr, lo):
                    for h in range(HPC):
                        nc.tensor.matmul(
                            ops[h][:, lo:W],
                            vaug[:, kt, h, :],
                            pr[:, h, lo:W],
                            start=(kt == 0),
                            stop=(kt == nkt - 1),
                        )

                pending = []
                for kt in range(nkt):
                    pending.append((kt, scores_exp(kt)))
                    if len(pending) > 2:
                        k0, (pr0, lo0) = pending.pop(0)
                        pv(k0, pr0, lo0)
                for k0, (pr0, lo0) in pending:
                    pv(k0, pr0, lo0)

                # normalize both heads through one reciprocal chain
                lrow = norm_pool.tile([1, HPC, W], FP32, tag="lrow")
                for h in range(HPC):
                    nc.vector.tensor_copy(lrow[:, h, :], ops[h][D : D + 1, :])
                lT = norm_pool.tile([128, HPC * W // 128], FP32, tag="lT")
                nc.gpsimd.dma_start(lT[:], lrow[:])
                rT = norm_pool.tile([128, HPC * W // 128], FP32, tag="rT")
                nc.vector.reciprocal(rT[:], lT[:])
                rrow = norm_pool.tile([1, HPC, W], FP32, tag="rrow")
                nc.gpsimd.dma_start(rrow[:], rT[:])
                bc = norm_pool.tile([D, HPC, W], FP32, tag="bc")
                nc.gpsimd.partition_broadcast(bc[:], rrow[:])
                for h in range(HPC):
                    nc.vector.tensor_mul(
                        outTt[h * D : (h + 1) * D, qsl0],
                        ops[h][0:D, :],
                        bc[:, h, :],
                    )

                if inline_outproj:
                    for tb in range(q0 // KT, (q0 + W) // KT):
                        emit_outproj_tb(b, tb)

        xt0 = emit_loads(0)
        emit_proj(0, xt0)
        emit_attention(0, inline_outproj=False)
        xt1 = emit_loads(1)
        emit_proj(1, xt1)
        for tb in range(T // KT):   # batch-0 outproj fills batch-1 attention
            emit_outproj_tb(0, tb)
        emit_attention(1, inline_outproj=True)

    nc.compile()
    return nc


def _get_nc():
    if not _NC_CACHE:
        _NC_CACHE.append(_build_nc())
    return _NC_CACHE[0]


def make_in_maps(x, w_qkv, w_proj):
    x = np.asarray(x, np.float32)
    w_qkv = np.asarray(w_qkv, np.float32)
    w_proj = np.asarray(w_proj, np.float32)
    xT = np.ascontiguousarray(x.transpose(2, 0, 1)).astype(NPBF)  # [E, B, T]
    in_maps = []
    for c in range(NCORES):
        h0 = c * HPC
        wq = w_qkv[h0 * D : (h0 + HPC) * D] * 0.125  # fold softmax scale
        wk = w_qkv[E + h0 * D : E + (h0 + HPC) * D]
        wv = w_qkv[2 * E + h0 * D : 2 * E + (h0 + HPC) * D]
        wqkvT = np.ascontiguousarray(np.concatenate([wq, wk, wv], 0).T)
        wpTc = np.ascontiguousarray(w_proj[:, c * JC : (c + 1) * JC].T)
        in_maps.append(
            {
                "xT": xT,
                "wqkvT": wqkvT.astype(NPBF),
                "wpT": wpTc.astype(NPBF),
            }
        )
    return in_maps


def kernel(x, w_qkv, w_proj, **run_kwargs):
    in_maps = make_in_maps(x, w_qkv, w_proj)
    nc = _get_nc()
    res = run_bass_kernel_spmd(nc, in_maps, core_ids=list(range(NCORES)), **run_kwargs)
    out = res.results[0]["outp"].astype(np.float32)
    for r in res.results[1:]:
        out += r["outp"].astype(np.float32)
    if run_kwargs:
        kernel.last_results = res
    return out


# revision 11
# speedup vs baseline: 1.0734x; 1.0236x over previous
"""Causal self-attention (B=2, T=2048, E=1024, H=16) on 8 trn2 NeuronCores.

Sharding: tensor-parallel over heads — core c owns heads {2c, 2c+1}.
Each core:
  1. qkv projection for its heads:  Q^T/K^T in [d, t] layout (d on
     partitions), V via PE-transpose into [t, d] layout.  For b=0 the
     ct loop is outermost (4 psum accumulators, 2 borrowed from the
     scores pool) so matmuls start as each x tile lands from HBM; b=1
     runs chunk-outer on the 2-slot proj pool since its x load hides
     under b=0's attention.
  2. causal attention with scores TRANSPOSED ([k, q] layout), processed
     in 512-query windows:
       scoresT(h0)/scoresT(h1) land in one [128, 2, 512] fp32 PSUM tile
       (2 banks); the two score matmuls are 64x128 row-tiles (T0 / T8)
       that the PE runs concurrently;
       ONE exp ACTIVATE per k-tile covers both heads;
       probsT  = exp * causal band mask (one paired mul on diag tiles)
       outT   += matmul(lhsT=V_aug, rhs=probsT)  (V_aug has a ones
                 column; its output row is the softmax denominator l)
       out     = outT[0:64] * (1/l)     (both heads share one
                 DMA-reshape / reciprocal / partition-broadcast chain)
  3. output projection: batch 0's is emitted AFTER batch 1's projection
     so the psum-slot rotation schedules it into batch 1's attention;
     batch 1's is emitted per window.  Partials are stored bf16 and
     summed on the host.

PSUM budget (8 banks): 2x scores [128,2,512] fp32 (4) + 2x [65,512] PV
accumulators (2) + 2x proj/outproj/transpose slots (2).  The PV pool
serves ONLY the accumulators: pool slots rotate in tile-creation order,
so sharing a pool across phases serializes the later phase behind the
earlier one.
"""

import numpy as np
import ml_dtypes
from contextlib import ExitStack

import concourse.bass as bass
import concourse.mybir as mybir
import concourse.tile as tile
from concourse import bacc
from concourse.bass_utils import run_bass_kernel_spmd
from concourse.masks import make_identity

B, T, E, H, D = 2, 2048, 1024, 16, 64
NCORES = 8
HPC = H // NCORES          # heads per core = 2
JC = HPC * D               # local out-projection columns per core = 128
W = 512                    # query window (one PSUM bank of fp32)
KT = 128                   # k tile (matmul M limit)
NW = T // W                # windows per head per batch = 4
NKT = T // KT              # k tiles = 16

BF16 = mybir.dt.bfloat16
FP32 = mybir.dt.float32
NPBF = ml_dtypes.bfloat16
EXP = mybir.ActivationFunctionType.Exp

_NC_CACHE = []


def _build_nc():
    nc = bacc.Bacc(None, target_bir_lowering=False)

    xT = nc.dram_tensor("xT", [E, B, T], BF16, kind="ExternalInput")
    wqkvT = nc.dram_tensor("wqkvT", [E, 3 * JC], BF16, kind="ExternalInput")
    wpT = nc.dram_tensor("wpT", [JC, E], BF16, kind="ExternalInput")
    outp = nc.dram_tensor("outp", [B, T, E], BF16, kind="ExternalOutput")

    with tile.TileContext(nc) as tc, ExitStack() as ctx:
        const_pool = ctx.enter_context(tc.tile_pool(name="const", bufs=1))
        w_pool = ctx.enter_context(tc.tile_pool(name="w", bufs=1))
        xt_pool = ctx.enter_context(tc.tile_pool(name="xt", bufs=2))
        qk_pool = ctx.enter_context(tc.tile_pool(name="qk", bufs=2))
        va_pool = ctx.enter_context(tc.tile_pool(name="va", bufs=2))
        vtmp_pool = ctx.enter_context(tc.tile_pool(name="vtmp", bufs=2))
        probs_pool = ctx.enter_context(tc.tile_pool(name="probs", bufs=8))
        outT_pool = ctx.enter_context(tc.tile_pool(name="outT", bufs=2))
        norm_pool = ctx.enter_context(tc.tile_pool(name="norm", bufs=4))
        stage_pool = ctx.enter_context(tc.tile_pool(name="stage", bufs=3))
        sc_ps = ctx.enter_context(tc.tile_pool(name="sc_ps", bufs=2, space="PSUM"))
        pv_ps = ctx.enter_context(tc.tile_pool(name="pv_ps", bufs=2, space="PSUM"))
        pj_ps = ctx.enter_context(tc.tile_pool(name="pj_ps", bufs=2, space="PSUM"))

        # --- constants -------------------------------------------------
        ident = const_pool.tile([128, 128], BF16)
        make_identity(nc, ident[:])

        # mask2[p, h, j] = 1 iff j >= p (causal band, copy per head)
        mask2 = const_pool.tile([128, HPC, KT], BF16)
        nc.gpsimd.memset(mask2[:], 1.0)
        nc.gpsimd.affine_select(
            out=mask2[:],
            in_=mask2[:],
            compare_op=mybir.AluOpType.is_ge,
            fill=0.0,
            base=0,
            channel_multiplier=-1,
            pattern=[[0, HPC], [1, KT]],
        )

        wq_sb = [
            w_pool.tile([128, 3 * JC], BF16, tag=f"wq{i}", name=f"wq{i}")
            for i in range(8)
        ]
        wp_sb = w_pool.tile([JC, E], BF16, tag="wp")
        nc.sync.dma_start(wp_sb[:], wpT[:])

        # vaug / outT for both batches upfront; memsets run before any
        # partition_broadcast so the gpsimd library loads only once.
        vaugs = [
            va_pool.tile([128, NKT, HPC, D + 1], BF16, tag="va", name=f"va{b}")
            for b in range(B)
        ]
        for b in range(B):
            nc.gpsimd.memset(vaugs[b][:, :, :, D : D + 1], 1.0)
        outTs = [
            outT_pool.tile([128, T], BF16, tag="outT", name=f"outT{b}")
            for b in range(B)
        ]
        qks = []

        def emit_loads(b):
            xt = [
                xt_pool.tile([128, T], BF16, tag=f"xt{i}", name=f"xt{i}")
                for i in range(8)
            ]
            for i in range(8):
                if b == 0:
                    nc.sync.dma_start(wq_sb[i][:], wqkvT[i * 128 : (i + 1) * 128, :])
                nc.sync.dma_start(xt[i][:], xT[i * 128 : (i + 1) * 128, b, :])
            return xt

        def emit_proj(b, xt):
            QT = qk_pool.tile([128, T], BF16, tag="QT", name=f"QT{b}")
            KTs = qk_pool.tile([128, T], BF16, tag="KT", name=f"KT{b}")
            qks.append((QT, KTs))
            vaug = vaugs[b]
            vtmps = [
                vtmp_pool.tile([128, W], BF16, tag=f"vtmp{c}", name=f"vtmp{c}")
                for c in range(4)
            ]

            def proj_out(fb, c, pp):
                tsl = slice(c * W, (c + 1) * W)
                if fb == 0:
                    nc.vector.tensor_copy(QT[:, tsl], pp[:])
                elif fb == 1:
                    nc.vector.tensor_copy(KTs[:, tsl], pp[:])
                else:
                    nc.vector.tensor_copy(vtmps[c][:], pp[:])

            def transposes():
                for kt_idx in range(16):
                    c, sub = divmod(kt_idx, W // KT)
                    ptr = pj_ps.tile([128, HPC, D], BF16, tag="pj_ps", name="ptr")
                    nc.tensor.transpose(
                        ptr[:], vtmps[c][:, sub * KT : (sub + 1) * KT], ident[:]
                    )
                    nc.vector.tensor_copy(vaug[:, kt_idx, :, 0:D], ptr[:])

            if b == 0:
                # ct-outer: all 4 chunks accumulate as each xt DMA lands
                for fb in (2, 0, 1):  # V, Q, K
                    pp = [
                        (sc_ps if c < 2 else pj_ps).tile(
                            [128, W], FP32, tag="sc_ps" if c < 2 else "pj_ps",
                            name=f"pp{c}",
                        )
                        for c in range(4)
                    ]
                    for ct in range(8):
                        for c in range(4):
                            nc.tensor.matmul(
                                pp[c][:],
                                wq_sb[ct][:, fb * 128 : (fb + 1) * 128],
                                xt[ct][:, c * W : (c + 1) * W],
                                start=(ct == 0),
                                stop=(ct == 7),
                            )
                    for c in range(4):
                        proj_out(fb, c, pp[c])
                transposes()
            else:
                for fb in (2, 0, 1):  # V, Q, K
                    for c in range(4):
                        pp = pj_ps.tile([128, W], FP32, tag="pj_ps", name="pp")
                        for ct in range(8):
                            nc.tensor.matmul(
                                pp[:],
                                wq_sb[ct][:, fb * 128 : (fb + 1) * 128],
                                xt[ct][:, c * W : (c + 1) * W],
                                start=(ct == 0),
                                stop=(ct == 7),
                            )
                        proj_out(fb, c, pp)
                    if fb == 2:
                        transposes()

        def emit_outproj_tb(b, tb):
            st = stage_pool.tile([128, E], BF16, tag="stage")
            for oc in range(2):
                pj = pj_ps.tile([128, W], FP32, tag="pj_ps", name="pj")
                nc.tensor.matmul(
                    pj[:],
                    outTs[b][:, tb * KT : (tb + 1) * KT],
                    wp_sb[:, oc * W : (oc + 1) * W],
                    start=True,
                    stop=True,
                )
                nc.vector.tensor_copy(st[:, oc * W : (oc + 1) * W], pj[:])
            nc.sync.dma_start(outp[b, tb * KT : (tb + 1) * KT, :], st[:])

        def emit_attention(b, inline_outproj, windows):
            QT, KTs = qks[b]
            vaug = vaugs[b]
            outTt = outTs[b]
            for wi in windows:
                q0 = wi * W
                nkt = (q0 + W) // KT
                qsl0 = slice(q0, q0 + W)

                ops = [
                    pv_ps.tile([D + 1, W], FP32, tag="pv_ps", name=f"ops{h}")
                    for h in range(HPC)
                ]

                def scores_exp(kt):
                    koff = kt * KT - q0
                    lo = max(0, koff)
                    ksl = slice(kt * KT, (kt + 1) * KT)
                    S = sc_ps.tile([128, HPC, W], FP32, tag="sc_ps", name="S")
                    pr = probs_pool.tile([128, HPC, W], BF16, tag="probs")
                    # 64x128 row tiles T0 / T8 — PE runs both heads
                    # concurrently.  High priority: whenever a scores
                    # matmul is ready the PE must prefer it over proj /
                    # outproj filler, else the exp chain (the critical
                    # ACT engine) starves.
                    with tc.high_priority():
                        for h in range(HPC):
                            po = h * D
                            nc.tensor.matmul(
                                S[:, h, lo:W],
                                KTs[po : po + D, ksl],
                                QT[po : po + D, q0 + lo : q0 + W],
                                start=True,
                                stop=True,
                            )
                    nc.scalar.activation(pr[:, :, lo:W], S[:, :, lo:W], EXP)
                    if koff >= 0:
                        nc.vector.tensor_mul(
                            pr[:, :, lo : lo + KT], pr[:, :, lo : lo + KT], mask2[:]
                        )
                    return pr, lo

                def pv(kt, pr, lo):
                    for h in range(HPC):
                        nc.tensor.matmul(
                            ops[h][:, lo:W],
                            vaug[:, kt, h, :],
                            pr[:, h, lo:W],
                            start=(kt == 0),
                            stop=(kt == nkt - 1),
                        )

                pending = []
                for kt in range(nkt):
                    pending.append((kt, scores_exp(kt)))
                    if len(pending) > 2:
                        k0, (pr0, lo0) = pending.pop(0)
                        pv(k0, pr0, lo0)
                for k0, (pr0, lo0) in pending:
                    pv(k0, pr0, lo0)

                # normalize both heads through one reciprocal chain
                lrow = norm_pool.tile([1, HPC, W], FP32, tag="lrow")
                for h in range(HPC):
                    nc.vector.tensor_copy(lrow[:, h, :], ops[h][D : D + 1, :])
                lT = norm_pool.tile([128, HPC * W // 128], FP32, tag="lT")
                nc.gpsimd.dma_start(lT[:], lrow[:])
                rT = norm_pool.tile([128, HPC * W // 128], FP32, tag="rT")
                nc.vector.reciprocal(rT[:], lT[:])
                rrow = norm_pool.tile([1, HPC, W], FP32, tag="rrow")
                nc.gpsimd.dma_start(rrow[:], rT[:])
                bc = norm_pool.tile([D, HPC, W], FP32, tag="bc")
                nc.gpsimd.partition_broadcast(bc[:], rrow[:])
                for h in range(HPC):
                    nc.vector.tensor_mul(
                        outTt[h * D : (h + 1) * D, qsl0],
                        ops[h][0:D, :],
                        bc[:, h, :],
                    )

                if inline_outproj:
                    for tb in range(q0 // KT, (q0 + W) // KT):
                        emit_outproj_tb(b, tb)

        xt0 = emit_loads(0)
        emit_proj(0, xt0)
        xt1 = emit_loads(1)          # runs during b0 attention
        emit_proj(1, xt1)            # fills b0-attention PE slack
        emit_attention(0, inline_outproj=False, windows=range(NW))
        for tb in range(T // KT):    # batch-0 outproj fills batch-1 attention
            emit_outproj_tb(0, tb)
        # big window first so the kernel tail is the smallest window
        emit_attention(1, inline_outproj=True, windows=range(NW - 1, -1, -1))

    nc.compile()
    return nc


def _get_nc():
    if not _NC_CACHE:
        _NC_CACHE.append(_build_nc())
    return _NC_CACHE[0]


def make_in_maps(x, w_qkv, w_proj):
    x = np.asarray(x, np.float32)
    w_qkv = np.asarray(w_qkv, np.float32)
    w_proj = np.asarray(w_proj, np.float32)
    xT = np.ascontiguousarray(x.transpose(2, 0, 1)).astype(NPBF)  # [E, B, T]
    in_maps = []
    for c in range(NCORES):
        h0 = c * HPC
        wq = w_qkv[h0 * D : (h0 + HPC) * D] * 0.125  # fold softmax scale
        wk = w_qkv[E + h0 * D : E + (h0 + HPC) * D]
        wv = w_qkv[2 * E + h0 * D : 2 * E + (h0 + HPC) * D]
        wqkvT = np.ascontiguousarray(np.concatenate([wq, wk, wv], 0).T)
        wpTc = np.ascontiguousarray(w_proj[:, c * JC : (c + 1) * JC].T)
        in_maps.append(
            {
                "xT": xT,
                "wqkvT": wqkvT.astype(NPBF),
                "wpT": wpTc.astype(NPBF),
            }
        )
    return in_maps


def kernel(x, w_qkv, w_proj, **run_kwargs):
    in_maps = make_in_maps(x, w_qkv, w_proj)
    nc = _get_nc()
    res = run_bass_kernel_spmd(nc, in_maps, core_ids=list(range(NCORES)), **run_kwargs)
    out = res.results[0]["outp"].astype(np.float32)
    for r in res.results[1:]:
        out += r["outp"].astype(np.float32)
    if run_kwargs:
        kernel.last_results = res
    return out
